# revision 1
# baseline (speedup 1.0000x reference)
"""Causal self-attention for Trainium2, 8-core SPMD (full-I/O contract).

Problem (hardcoded):
    x:     (4, 2048, 1024) f32
    w_qkv: (1024, 3072) f32   (cols = [q | k | v], each 1024 = 16 heads x 64)
    b_qkv: (3072,) f32
    w_out: (1024, 1024) f32
    b_out: (1024,) f32
    out = out_proj(causal_mha(x)), 16 heads, head_dim 64.

Sharding: batch(4) x head-group(2x8 heads) across 8 cores, Megatron-style.
Each core computes a partial (2048, 1024) output for one batch from its 8
heads; the host sums the two head-group partials per batch and adds the
biases that commute through softmax (b_v, b_out fold into a host-side row
bias; b_qk is applied on-device).

Per-core layout strategy (no transposes anywhere on device):
  - host supplies xT = x[b].T, so SBUF holds x with d on partitions
  - q,k are produced transposed (qkT: [qk_col, s]) via out.T = w.T @ x
  - v is produced natural ([s, v_col]) via out = xT.T @ w_v, interleaved
    with 32 ones-columns per head so the PV matmul emits the softmax
    denominator (replicated over 32 partitions) for free
  - scores are computed as S.T [sk, sq] (both operands direct qkT slices),
    exp on ScalarE with fused 1/sqrt(hd) scale, causal diagonal masked by a
    precomputed additive [128,128] mask; the PV matmul's column-range
    restriction implements the block causal structure
  - the attention result aT [d_slice, s] (bf16) is directly the stationary
    operand of the output projection, which emits y in natural layout for
    contiguous DMA out.
"""

import math
from contextlib import ExitStack

import numpy as np

import concourse.bacc as bacc
import concourse.bass as bass
import concourse.mybir as mybir
import concourse.tile as tile
from concourse.bass import ds

F32 = mybir.dt.float32
F32R = mybir.dt.float32r
BF16 = mybir.dt.bfloat16
AF = mybir.ActivationFunctionType
ALU = mybir.AluOpType

P = 128
NEG = -1e9  # additive causal-mask value (exp -> 0 in f32)


class Cfg:
    def __init__(self, S=2048, D=1024, NH=8, HD=64, DOUT=1024,
                 mm_dt="float32r", reps=1):
        assert HD == 64 and S % 512 == 0 and D % P == 0
        assert (2 * NH * HD) % 256 == 0 and (NH * HD) % P == 0
        assert DOUT % 256 == 0
        self.S, self.D, self.NH, self.HD, self.DOUT = S, D, NH, HD, DOUT
        self.mm_dt = F32R if mm_dt == "float32r" else F32
        self.reps = reps
        self.KC = D // P             # contraction chunks for projections
        self.QKC = 2 * NH * HD // P  # qkT column chunks (q cols then k cols)
        self.SC = S // 512           # 512-wide s chunks
        self.SC128 = S // P          # 128-wide s chunks
        self.VW = NH * HD            # v columns (natural layout width)
        self.DSL = NH * HD // P      # out-proj contraction chunks
        self.OW = DOUT // 2          # out-proj free width per matmul
        assert self.OW <= 512


def build_attn_nc(cfg: Cfg):
    """Build + compile the per-core Bass/Tile program."""
    nc = bacc.Bacc("TRN2", target_bir_lowering=False, debug=False)
    S, D, NH, HD = cfg.S, cfg.D, cfg.NH, cfg.HD

    MMD = cfg.mm_dt
    xT = nc.dram_tensor("xt", [D, S], MMD, kind="ExternalInput").ap()
    w_qk = nc.dram_tensor("w_qk", [D, 2 * NH * HD], MMD, kind="ExternalInput").ap()
    w_v = nc.dram_tensor("w_v", [D, NH * HD], MMD, kind="ExternalInput").ap()
    b_qk = nc.dram_tensor("b_qk", [2 * NH * HD], F32, kind="ExternalInput").ap()
    w_out = nc.dram_tensor("w_out", [NH * HD, cfg.DOUT], BF16,
                           kind="ExternalInput").ap()
    yp = nc.dram_tensor("yp", [S, cfg.DOUT], F32, kind="ExternalOutput").ap()

    def mm(out, lhsT, rhs, start, stop):
        nc.tensor.matmul(out, lhsT, rhs, start=start, stop=stop)

    with tile.TileContext(nc) as tc:
        with ExitStack() as ctx:
            body = _emit(ctx, nc, tc, cfg, xT, w_qk, w_v, b_qk, w_out, yp, mm)
            if cfg.reps == 1:
                body()
            else:
                with tc.For_i(0, cfg.reps, 1):
                    body()

    nc.compile()
    return nc


def _emit(ctx, nc, tc, cfg, xT, w_qk, w_v, b_qk, w_out, yp, mm):
    S, D, NH, HD = cfg.S, cfg.D, cfg.NH, cfg.HD
    KC, QKC, SC, SC128 = cfg.KC, cfg.QKC, cfg.SC, cfg.SC128
    VW, DSL, OW = cfg.VW, cfg.DSL, cfg.OW
    HV = HD + 32  # v cols + 32 ones-cols (denominator replicas) per head
    MMD = cfg.mm_dt

    pool = lambda name, bufs, **kw: ctx.enter_context(
        tc.tile_pool(name=name, bufs=bufs, **kw))

    cst = pool("cst", 1)
    qkT_pool = pool("qkT", 1)
    vbuf_pool = pool("vbuf", 1)
    aT_pool = pool("aT", 1)
    pt_pool = pool("pt", 2)
    rc_pool = pool("rc", 2)
    wv_pool = pool("wv", 1)
    ps = pool("ps", 3, space="PSUM")
    pso = pool("pso", 2, space="PSUM")

    def body():
        # ---------- constants / weights ----------
        mask = cst.tile([P, P], F32, tag="mask")
        nc.gpsimd.memset(mask[:], 0.0)
        # S.T layout: keep 0 where (col - row) >= 0, else NEG
        nc.gpsimd.affine_select(
            out=mask[:], in_=mask[:], compare_op=ALU.is_ge, fill=NEG,
            base=0, pattern=[[1, P]], channel_multiplier=-1,
        )
        bqk_t = cst.tile([P, QKC], F32, tag="bqk")
        nc.sync.dma_start(bqk_t[:], b_qk.rearrange("(c p) -> p c", p=P))

        wv_t = wv_pool.tile([P, KC, VW], MMD, tag="wv")
        nc.sync.dma_start(wv_t[:], w_v.rearrange("(c p) v -> p c v", p=P))

        # ---------- persistent activations ----------
        qkT = qkT_pool.tile([P, QKC, S], MMD, tag="qkT")
        vb = vbuf_pool.tile([P, SC128, NH, HV], MMD, tag="vbuf")
        aT = aT_pool.tile([P, DSL, S], BF16, tag="aT")
        nc.vector.memset(vb[:, :, :, HD:HV].bitcast(F32), 1.0)

        inv_sqrt_hd = 1.0 / math.sqrt(HD)

        def proj_chunk(j):
            xts = xts_pool.tile([P, KC, 512], MMD, tag="xts")
            nc.sync.dma_start(
                xts[:],
                xT.rearrange("(c p) s -> p c s", p=P)[:, :, ds(j * 512, 512)])
            # q,k in out.T orientation -> qkT[c, s]; c-chunks paired
            for cg in range(QKC // 2):
                wq = wqk_pool.tile([P, KC, 256], MMD, tag="wqk")
                nc.sync.dma_start(
                    wq[:],
                    w_qk.rearrange("(c p) n -> p c n", p=P)[
                        :, :, ds(cg * 256, 256)])
                ps_qk = ps.tile([P, 1024], F32, tag="psA")
                for k in range(KC):
                    for ch in range(2):
                        mm(ps_qk[:, ds(ch * 512, 512)],
                           wq[:, k, ds(ch * P, P)], xts[:, k, :],
                           start=(k == 0), stop=(k == KC - 1))
                for ch in range(2):
                    c = cg * 2 + ch
                    nc.vector.tensor_scalar_add(
                        qkT[:, c, ds(j * 512, 512)],
                        ps_qk[:, ds(ch * 512, 512)], bqk_t[:, ds(c, 1)])
            # v in natural orientation -> vb[s, v_col]; s-subchunks paired
            for sp in range(2):
                ps_v = ps.tile([P, 1024], F32, tag="psA")
                for k in range(KC):
                    for half in range(2):
                        mm(ps_v[:, ds(half * 512, VW)],
                           xts[:, k, ds((sp * 2 + half) * P, P)],
                           wv_t[:, k, :],
                           start=(k == 0), stop=(k == KC - 1))
                for half in range(2):
                    sc = j * 4 + sp * 2 + half
                    nc.vector.tensor_copy(
                        vb[:, sc, :, 0:HD],
                        ps_v[:, ds(half * 512, VW)].rearrange(
                            "p (h c) -> p h c", c=HD))

        def attn_chunk(j):
            n_t = 4 * j + 4  # sk tiles for this sq chunk (always even)
            for hp in range(NH // 2):
                for hh in range(2):
                    h = hp * 2 + hh
                    boff = (h % 2) * HD
                    cq, ck = h // 2, QKC // 2 + h // 2
                    ps_o = pso.tile([P, 512], F32, tag="psO")
                    o_sl = ps_o[:]
                    qT = qkT[ds(boff, HD), cq, ds(j * 512, 512)]
                    for pr in range(n_t // 2):
                        ps_s = ps.tile([P, 1024], F32, tag="psA")
                        for sl in range(2):
                            t = pr * 2 + sl
                            mm(ps_s[:, ds(sl * 512, 512)],
                               qkT[ds(boff, HD), ck, ds(t * P, P)], qT,
                               start=True, stop=True)
                        for sl in range(2):
                            t = pr * 2 + sl
                            if t >= 4 * j:  # diagonal-band tile: mask diag block
                                u = t - 4 * j
                                nc.vector.tensor_tensor(
                                    ps_s[:, ds(sl * 512 + u * P, P)],
                                    ps_s[:, ds(sl * 512 + u * P, P)],
                                    mask[:], ALU.add)
                        pt = pt_pool.tile([P, 1024], MMD, tag="pt")
                        # exp only from the first valid column of the pair on
                        n0p = max(0, (pr * 2 - 4 * j)) * P
                        nc.scalar.activation(pt[:, ds(n0p, 1024 - n0p)],
                                             ps_s[:, ds(n0p, 1024 - n0p)],
                                             AF.Exp, scale=inv_sqrt_hd)
                        for sl in range(2):
                            t = pr * 2 + sl
                            n0 = 0 if t < 4 * j else (t - 4 * j) * P
                            mm(o_sl[ds(0, HD + 32), ds(n0, 512 - n0)],
                               vb[:, t, h, :],
                               pt[:, ds(sl * 512 + n0, 512 - n0)],
                               start=(t == 0), stop=(t == n_t - 1))
                    # rows [HD, HD+32) of o_sl hold the replicated denominator
                    rc = rc_pool.tile([32, 512], F32, tag="rc")
                    nc.vector.reciprocal(rc[:], o_sl[ds(HD, 32), :])
                    for qd in range(2):
                        nc.vector.tensor_tensor(
                            aT[ds(boff + qd * 32, 32), h // 2, ds(j * 512, 512)],
                            o_sl[ds(qd * 32, 32), :], rc[:], ALU.mult)

        # phases 1+2 interleaved per s-chunk so ScalarE exp overlaps PE proj
        with tc.tile_pool(name="xts", bufs=2) as xts_pool, \
                tc.tile_pool(name="wqk", bufs=2) as wqk_pool:
            for j in range(SC):
                proj_chunk(j)
                attn_chunk(j)

        # ---------- phase 3: output projection ----------
        with tc.tile_pool(name="wout", bufs=1) as wout_pool, \
                tc.tile_pool(name="yo", bufs=2) as yo_pool:
            wout_t = wout_pool.tile([P, DSL, cfg.DOUT], BF16, tag="wout")
            nc.sync.dma_start(wout_t[:],
                              w_out.rearrange("(c p) d -> p c d", p=P))
            for sc in range(SC128):
                ps_y = ps.tile([P, 2 * OW], F32, tag="psA")
                for k in range(DSL):
                    for g in range(2):
                        mm(ps_y[:, ds(g * OW, OW)],
                           aT[:, k, ds(sc * P, P)],
                           wout_t[:, k, ds(g * OW, OW)],
                           start=(k == 0), stop=(k == DSL - 1))
                y_t = yo_pool.tile([P, 2 * OW], F32, tag="yo")
                nc.scalar.copy(y_t[:], ps_y[:])
                nc.sync.dma_start(yp[ds(sc * P, P), :], y_t[:])

    return body


# ---------------------------------------------------------------------------
# Host-side runner: shard full inputs, run 8-core SPMD, gather + reduce.
# ---------------------------------------------------------------------------

_RUNNER_CACHE = {}


class _Runner:
    def __init__(self, cfg: Cfg):
        import jax
        from jax.experimental.shard_map import shard_map
        from jax.sharding import Mesh, NamedSharding, PartitionSpec

        from concourse import bass2jax

        self.cfg = cfg
        self.nc = build_attn_nc(cfg)
        nc = self.nc
        bass2jax.install_neuronx_cc_hook()

        part_name = nc.partition_id_tensor.name if nc.partition_id_tensor else None
        in_names, out_names, out_avals = [], [], []
        for alloc in nc.m.functions[0].allocations:
            if not isinstance(alloc, mybir.MemoryLocationSet):
                continue
            name = alloc.memorylocations[0].name
            if alloc.kind == "ExternalInput":
                if name != part_name:
                    in_names.append(name)
            elif alloc.kind == "ExternalOutput":
                out_names.append(name)
                out_avals.append(jax.core.ShapedArray(
                    tuple(alloc.tensor_shape), mybir.dt.np(alloc.dtype)))
        self.in_names, self.out_names = in_names, out_names
        all_names = in_names + out_names + ([part_name] if part_name else [])
        n_params = len(in_names)

        def _body(*args):
            operands = list(args)
            if part_name:
                operands.append(bass2jax.partition_id_tensor())
            return tuple(bass2jax._bass_exec_p.bind(
                *operands, out_avals=tuple(out_avals),
                in_names=tuple(all_names), out_names=tuple(out_names),
                lowering_input_output_aliases=(),
                sim_require_finite=True, sim_require_nnan=True, nc=nc))

        n_cores = 8
        devices = jax.devices()[:n_cores]
        mesh = Mesh(np.asarray(devices), ("core",))
        self.sharding = NamedSharding(mesh, PartitionSpec("core"))
        n_out = len(out_names)
        self.fn = jax.jit(
            shard_map(_body, mesh=mesh,
                      in_specs=(PartitionSpec("core"),) * (n_params + n_out),
                      out_specs=(PartitionSpec("core"),) * n_out,
                      check_rep=False),
            donate_argnums=tuple(range(n_params, n_params + n_out)),
            keep_unused=True)
        self.out_avals = out_avals
        self._jax = jax

    def run(self, per_core_inputs):
        """per_core_inputs: list of 8 dicts keyed by bass input name."""
        import jax.numpy as jnp
        jax = self._jax
        concat_in = [
            np.concatenate([np.asarray(per_core_inputs[c][n])
                            for c in range(8)], axis=0)
            for n in self.in_names
        ]
        dev_in = [jax.device_put(a, self.sharding) for a in concat_in]
        zeros = [jnp.zeros((8 * av.shape[0], *av.shape[1:]), av.dtype,
                           device=self.sharding) for av in self.out_avals]
        outs = self.fn(*dev_in, *zeros)
        outs = [np.asarray(o) for o in outs]
        return [
            {n: outs[i].reshape(8, *self.out_avals[i].shape)[c]
             for i, n in enumerate(self.out_names)}
            for c in range(8)
        ]


def get_runner(reps=1, mm_dt="float32r"):
    key = (reps, mm_dt)
    if key not in _RUNNER_CACHE:
        _RUNNER_CACHE[key] = _Runner(Cfg(mm_dt=mm_dt, reps=reps))
    return _RUNNER_CACHE[key]


def shard_inputs(x, w_qkv, b_qkv, w_out, b_out):
    """Full inputs -> 8 per-core input dicts (core c = batch c//2, hgroup c%2)."""
    import ml_dtypes
    x = np.asarray(x, np.float32)
    w_qkv = np.asarray(w_qkv, np.float32)
    b_qkv = np.asarray(b_qkv, np.float32)
    w_out = np.asarray(w_out, np.float32)
    xTs = [np.ascontiguousarray(x[b].T) for b in range(x.shape[0])]
    per_core = []
    for c in range(8):
        b, hg = c // 2, c % 2
        q_sl = slice(hg * 512, hg * 512 + 512)
        k_sl = slice(1024 + hg * 512, 1024 + hg * 512 + 512)
        v_sl = slice(2048 + hg * 512, 2048 + hg * 512 + 512)
        per_core.append({
            "xt": xTs[b],
            "w_qk": np.ascontiguousarray(
                np.concatenate([w_qkv[:, q_sl], w_qkv[:, k_sl]], axis=1)),
            "w_v": np.ascontiguousarray(w_qkv[:, v_sl]),
            "b_qk": np.ascontiguousarray(
                np.concatenate([b_qkv[q_sl], b_qkv[k_sl]])),
            "w_out": np.ascontiguousarray(
                w_out[hg * 512:(hg + 1) * 512, :]).astype(ml_dtypes.bfloat16),
        })
    return per_core


def kernel(x, w_qkv, b_qkv, w_out, b_out):
    runner = get_runner()
    per_core = shard_inputs(x, w_qkv, b_qkv, w_out, b_out)
    results = runner.run(per_core)
    b_v = np.asarray(b_qkv, np.float32)[2048:]
    bias = np.asarray(b_out, np.float32) + b_v @ np.asarray(w_out, np.float32)
    out = np.empty((4, 2048, 1024), np.float32)
    for b in range(4):
        out[b] = results[2 * b]["yp"] + results[2 * b + 1]["yp"] + bias
    return out



# revision 15
# speedup vs baseline: 1.4664x; 1.4664x over previous
"""Causal self-attention for Trainium2, 8-core SPMD (full-I/O contract).

Problem (hardcoded):
    x:     (4, 2048, 1024) f32
    w_qkv: (1024, 3072) f32   (cols = [q | k | v], each 1024 = 16 heads x 64)
    b_qkv: (3072,) f32
    w_out: (1024, 1024) f32
    b_out: (1024,) f32
    out = out_proj(causal_mha(x)), 16 heads, head_dim 64.

Sharding: batch(4) x head-group(2x8 heads) across 8 cores, Megatron-style.
Each core computes a partial (2048, 1024) output for one batch from its 8
heads; the host sums the two head-group partials per batch and adds the
biases that commute through softmax (b_v, b_out fold into a host-side row
bias; b_qk is applied on-device).

Per-core layout strategy (no transposes anywhere on device):
  - host supplies xT = x[b].T in bf16, so SBUF holds x with d on partitions
  - q,k are produced transposed (qkT: [qk_col, s]) via out.T = w.T @ x
  - v is produced natural ([s, v_col]) via out = xT.T @ w_v, interleaved
    with 32 ones-columns per head so the PV matmul emits the softmax
    denominator (replicated over 32 partitions) for free
  - scores are computed as S.T [sk, sq] (both operands direct qkT slices),
    exp on ScalarE with fused 1/sqrt(hd) scale, causal diagonal masked by a
    precomputed additive [128,128] mask; the PV matmul's column-range
    restriction implements the block causal structure
  - the attention result aT [d_slice, s] (bf16) is directly the stationary
    operand of the output projection, which emits y (bf16) in natural
    layout for contiguous DMA out.

Scheduling: all matmuls bf16 (1 col/cycle on PE). The per-pair chain
scores(PE) -> mask(DVE) -> exp(ScalarE) -> PV(PE) would stall PE on exp
latency, so (a) scores for pair p+1 are issued before PV of pair p, and
(b) the projection work for s-chunk j+1 (and, in the last chunk, the
output projection for the already-complete s range) is chopped into
~0.9us matmul quanta and drained between attention pairs of chunk j,
giving PE independent work whenever the softmax pipeline lags.
"""

import math
from contextlib import ExitStack

import numpy as np

import concourse.bacc as bacc
import concourse.bass as bass
import concourse.mybir as mybir
import concourse.tile as tile
from concourse.bass import ds

F32 = mybir.dt.float32
F32R = mybir.dt.float32r
BF16 = mybir.dt.bfloat16
AF = mybir.ActivationFunctionType
ALU = mybir.AluOpType

P = 128
NEG = -1e9  # additive causal-mask value (exp -> 0 in f32)


class Cfg:
    def __init__(self, S=2048, D=1024, NH=8, HD=64, DOUT=1024,
                 mm_dt="bfloat16", reps=1):
        assert HD == 64 and S % 512 == 0 and D % P == 0
        assert (2 * NH * HD) % 256 == 0 and (NH * HD) % P == 0
        assert DOUT % 256 == 0
        self.S, self.D, self.NH, self.HD, self.DOUT = S, D, NH, HD, DOUT
        self.mm_dt = {"float32r": F32R, "bfloat16": BF16,
                      "float32": F32}[mm_dt]
        self.reps = reps
        self.KC = D // P             # contraction chunks for projections
        self.QKC = 2 * NH * HD // P  # qkT column chunks (q cols then k cols)
        self.SC = S // 512           # 512-wide s chunks
        self.SC128 = S // P          # 128-wide s chunks
        self.VW = NH * HD            # v columns (natural layout width)
        self.DSL = NH * HD // P      # out-proj contraction chunks
        self.OW = DOUT // 2          # out-proj free width per matmul
        assert self.OW <= 512


def build_attn_nc(cfg: Cfg):
    """Build + compile the per-core Bass/Tile program."""
    nc = bacc.Bacc("TRN2", target_bir_lowering=False, debug=False)
    S, D, NH, HD = cfg.S, cfg.D, cfg.NH, cfg.HD

    MMD = cfg.mm_dt
    xT = nc.dram_tensor("xt", [D, S], MMD, kind="ExternalInput").ap()
    w_qk = nc.dram_tensor("w_qk", [D, 2 * NH * HD], MMD, kind="ExternalInput").ap()
    w_v = nc.dram_tensor("w_v", [D, NH * HD], MMD, kind="ExternalInput").ap()
    b_qk = nc.dram_tensor("b_qk", [2 * NH * HD], F32, kind="ExternalInput").ap()
    w_out = nc.dram_tensor("w_out", [NH * HD, cfg.DOUT], BF16,
                           kind="ExternalInput").ap()
    yp = nc.dram_tensor("yp", [S, cfg.DOUT], BF16, kind="ExternalOutput").ap()

    def mm(out, lhsT, rhs, start, stop):
        nc.tensor.matmul(out, lhsT, rhs, start=start, stop=stop)

    with tile.TileContext(nc) as tc:
        with ExitStack() as ctx:
            body = _emit(ctx, nc, tc, cfg, xT, w_qk, w_v, b_qk, w_out, yp, mm)
            if cfg.reps == 1:
                body()
            else:
                with tc.For_i(0, cfg.reps, 1):
                    body()

    nc.compile()
    return nc


def _emit(ctx, nc, tc, cfg, xT, w_qk, w_v, b_qk, w_out, yp, mm):
    S, D, NH, HD = cfg.S, cfg.D, cfg.NH, cfg.HD
    KC, QKC, SC, SC128 = cfg.KC, cfg.QKC, cfg.SC, cfg.SC128
    VW, DSL, OW = cfg.VW, cfg.DSL, cfg.OW
    HV = 2 * HD  # v cols + 64 ones-cols (denominator replicas) per head
    MMD = cfg.mm_dt

    pool = lambda name, bufs, **kw: ctx.enter_context(
        tc.tile_pool(name=name, bufs=bufs, **kw))

    cst = pool("cst", 1)
    qkT_pool = pool("qkT", 1)
    vbuf_pool = pool("vbuf", 1)
    aT_pool = pool("aT", 1)
    pt_pool = pool("pt", 4)
    rc_pool = pool("rc", 3)
    wv_pool = pool("wv", 1)
    wout_pool = pool("wout", 1)
    xts_pool = pool("xts", 2)
    wqk_pool = pool("wqk", 3)
    yo_pool = pool("yo", 2)
    ps = pool("ps", 2, space="PSUM")      # scores pairs: 2-bank tiles
    psb = pool("psb", 2, space="PSUM")    # proj/out-proj accum: 1-bank tiles
    pso = pool("pso", 2, space="PSUM")

    xr = xT.rearrange("(c p) s -> p c s", p=P)
    wqr = w_qk.rearrange("(c p) n -> p c n", p=P)

    def body():
        inv_sqrt_hd = 1.0 / math.sqrt(HD)

        # ------------------------------------------------------------------
        # Projection chunk j as a list of small PE quanta (closures).
        # ------------------------------------------------------------------
        def proj_quanta(j):
            st = {}

            half = QKC // 2
            # emission order: q-block i, k-block i, v tile i -- head i*2's
            # attention deps (q, k) and the first PV's v tiles finish first
            cseq = []
            for i in range(half):
                cseq += [i, half + i]

            def load_wq(c):
                w = wqk_pool.tile([P, KC, P], MMD, tag="wqk")
                st[("wq", c)] = w
                nc.sync.dma_start(w[:], wqr[:, :, ds(c * P, P)])

            def q_prefetch():
                xts = xts_pool.tile([P, KC, 512], MMD, tag="xts")
                st["xts"] = xts
                for kk in range(0, KC, 2):
                    nc.sync.dma_start(
                        xts[:, kk:kk + 2, :],
                        xr[:, kk:kk + 2, ds(j * 512, 512)])
                load_wq(cseq[0])
                load_wq(cseq[1])

            def mk_qk(i):
                c = cseq[i]
                def q():
                    if i + 2 < QKC:
                        load_wq(cseq[i + 2])
                    ps_qk = psb.tile([P, 512], F32, tag="psB")
                    wq = st.pop(("wq", c))
                    xts = st["xts"]
                    for k in range(KC):
                        mm(ps_qk[:], wq[:, k, :], xts[:, k, :],
                           start=(k == 0), stop=(k == KC - 1))
                    nc.vector.tensor_scalar_add(
                        qkT[:, c, ds(j * 512, 512)], ps_qk[:],
                        bqk_t[:, ds(c, 1)])
                return q

            def mk_v(sq):
                # one 128-row s tile of v: alloc, 8 matmuls, copy, release
                def q():
                    ps_v = psb.tile([P, 512], F32, tag="psB")
                    xts = st["xts"]
                    for k in range(KC):
                        mm(ps_v[:], xts[:, k, ds(sq * P, P)], wv_t[:, k, :],
                           start=(k == 0), stop=(k == KC - 1))
                    nc.vector.tensor_copy(
                        vb[:, j * 4 + sq, :, 0:HD],
                        ps_v[:].rearrange("p (h c) -> p h c", c=HD))
                return q

            qs = [q_prefetch]
            for i in range(half):
                qs.append(mk_qk(2 * i))
                qs.append(mk_qk(2 * i + 1))
                qs.append(mk_v(i))
            return qs

        # ------------------------------------------------------------------
        # Output projection for one 128-row s tile, as 2 quanta.
        # ------------------------------------------------------------------
        wout_st = {}

        def q_wout_prefetch():
            wout_t = wout_pool.tile([P, DSL, cfg.DOUT], BF16, tag="wout")
            wout_st["w"] = wout_t
            nc.sync.dma_start(wout_t[:],
                              w_out.rearrange("(c p) d -> p c d", p=P))

        def outproj_quanta(sc):
            st = {}

            def mk(g):
                def q():
                    ps_y = psb.tile([P, OW], F32, tag="psB")
                    wout_t = wout_st["w"]
                    for k in range(DSL):
                        mm(ps_y[:], aT[:, k, ds(sc * P, P)],
                           wout_t[:, k, ds(g * OW, OW)],
                           start=(k == 0), stop=(k == DSL - 1))
                    if g == 0:
                        y_t = yo_pool.tile([P, 2 * OW], BF16, tag="yo")
                        st["y"] = y_t
                    y_t = st["y"]
                    nc.vector.tensor_copy(y_t[:, ds(g * OW, OW)], ps_y[:])
                    if g == 1:
                        nc.sync.dma_start(yp[ds(sc * P, P), :], y_t[:])
                return q

            return [mk(0), mk(1)]

        # ------------------------------------------------------------------
        # Attention chunk j; drains `extras` quanta between pairs.
        # ------------------------------------------------------------------
        def attn_chunk(j, extras):
            n_t = 4 * j + 4  # sk tiles for this sq chunk (always even)
            n_pr = n_t // 2
            total_pairs = NH * n_pr
            drained = 0
            pair_idx = 0

            def drain(target):
                nonlocal drained
                while drained < target:
                    extras[drained]()
                    drained += 1

            for h in range(NH):
                boff = (h % 2) * HD
                cq, ck = h // 2, QKC // 2 + h // 2
                ps_o = pso.tile([P, 512], F32, tag="psO")
                o_sl = ps_o[:]
                qT = qkT[ds(boff, HD), cq, ds(j * 512, 512)]
                pts = {}

                def emit_scores(pr):
                    n0p = max(0, pr * 2 - 4 * j) * P
                    ps_s = ps.tile([P, 1024], F32, tag="psS")
                    for sl in range(2):
                        t = pr * 2 + sl
                        mm(ps_s[:, ds(sl * 512 + n0p, 512 - n0p)],
                           qkT[ds(boff, HD), ck, ds(t * P, P)],
                           qT[:, ds(n0p, 512 - n0p)],
                           start=True, stop=True)
                    for sl in range(2):
                        t = pr * 2 + sl
                        if t >= 4 * j:  # diagonal-band tile: mask diag block
                            u = t - 4 * j
                            nc.vector.tensor_tensor(
                                ps_s[:, ds(sl * 512 + u * P, P)],
                                ps_s[:, ds(sl * 512 + u * P, P)],
                                mask[:], ALU.add)
                    pt = pt_pool.tile([P, 1024], MMD, tag="pt")
                    nc.scalar.activation(pt[:, ds(n0p, 1024 - n0p)],
                                         ps_s[:, ds(n0p, 1024 - n0p)],
                                         AF.Exp, scale=inv_sqrt_hd)
                    pts[pr] = pt

                def emit_pv(pr):
                    pt = pts.pop(pr)
                    for sl in range(2):
                        t = pr * 2 + sl
                        n0 = 0 if t < 4 * j else (t - 4 * j) * P
                        mm(o_sl[:, ds(n0, 512 - n0)],
                           vb[:, t, h, :],
                           pt[:, ds(sl * 512 + n0, 512 - n0)],
                           start=(t == 0), stop=(t == n_t - 1))

                for pr in range(n_pr):
                    emit_scores(pr)
                    if pr > 0:
                        emit_pv(pr - 1)
                    pair_idx += 1
                    drain(pair_idx * len(extras) // total_pairs)
                emit_pv(n_pr - 1)

                # rows [HD, 2*HD) of o_sl hold the replicated denominator
                rc = rc_pool.tile([HD, 512], F32, tag="rc")
                nc.vector.reciprocal(rc[:], o_sl[ds(HD, HD), :])
                nc.vector.tensor_tensor(
                    aT[ds(boff, HD), h // 2, ds(j * 512, 512)],
                    o_sl[ds(0, HD), :], rc[:], ALU.mult)
            drain(len(extras))

        # ------------------------------------------------------------------
        # Schedule: proj(0); attn(j) x proj(j+1); attn(3) x outproj(0..11);
        # outproj tail.
        # ------------------------------------------------------------------
        qs0 = proj_quanta(0)
        qs0[0]()  # chunk-0 x/wq prefetch: first DMAs on the queue

        # constants / weights / persistent activations (DMAs issued after
        # the chunk-0 prefetch so the first matmuls start early)
        mask = cst.tile([P, P], F32, tag="mask")
        nc.gpsimd.memset(mask[:], 0.0)
        # S.T layout: keep 0 where (col - row) >= 0, else NEG
        nc.gpsimd.affine_select(
            out=mask[:], in_=mask[:], compare_op=ALU.is_ge, fill=NEG,
            base=0, pattern=[[1, P]], channel_multiplier=-1,
        )
        bqk_t = cst.tile([P, QKC], F32, tag="bqk")
        nc.sync.dma_start(bqk_t[:], b_qk.rearrange("(c p) -> p c", p=P))
        wv_t = wv_pool.tile([P, KC, VW], MMD, tag="wv")
        nc.sync.dma_start(wv_t[:], w_v.rearrange("(c p) v -> p c v", p=P))
        qkT = qkT_pool.tile([P, QKC, S], MMD, tag="qkT")
        vb = vbuf_pool.tile([P, SC128, NH, HV], MMD, tag="vbuf")
        aT = aT_pool.tile([P, DSL, S], BF16, tag="aT")
        if MMD == BF16:
            nc.vector.memset(vb[:, :, :, HD:HV], 1.0)
        else:
            nc.vector.memset(vb[:, :, :, HD:HV].bitcast(F32), 1.0)

        for q in qs0[1:]:
            q()
        n_early = SC128 - 4  # out-proj tiles whose aT is ready before attn(3)
        for j in range(SC):
            if j < SC - 1:
                extras = proj_quanta(j + 1)
                if j == SC - 2:
                    extras = extras + [q_wout_prefetch]
            else:
                extras = []
                for sc in range(n_early):
                    extras += outproj_quanta(sc)
            attn_chunk(j, extras)
        for sc in range(n_early, SC128):
            for q in outproj_quanta(sc):
                q()

    return body


# ---------------------------------------------------------------------------
# Host-side runner: shard full inputs, run 8-core SPMD, gather + reduce.
# ---------------------------------------------------------------------------

_RUNNER_CACHE = {}


class _Runner:
    def __init__(self, cfg: Cfg):
        import jax
        from jax.experimental.shard_map import shard_map
        from jax.sharding import Mesh, NamedSharding, PartitionSpec

        from concourse import bass2jax

        self.cfg = cfg
        self.nc = build_attn_nc(cfg)
        nc = self.nc
        bass2jax.install_neuronx_cc_hook()

        part_name = nc.partition_id_tensor.name if nc.partition_id_tensor else None
        in_names, out_names, out_avals = [], [], []
        for alloc in nc.m.functions[0].allocations:
            if not isinstance(alloc, mybir.MemoryLocationSet):
                continue
            name = alloc.memorylocations[0].name
            if alloc.kind == "ExternalInput":
                if name != part_name:
                    in_names.append(name)
            elif alloc.kind == "ExternalOutput":
                out_names.append(name)
                out_avals.append(jax.core.ShapedArray(
                    tuple(alloc.tensor_shape), mybir.dt.np(alloc.dtype)))
        self.in_names, self.out_names = in_names, out_names
        all_names = in_names + out_names + ([part_name] if part_name else [])
        n_params = len(in_names)

        def _body(*args):
            operands = list(args)
            if part_name:
                operands.append(bass2jax.partition_id_tensor())
            return tuple(bass2jax._bass_exec_p.bind(
                *operands, out_avals=tuple(out_avals),
                in_names=tuple(all_names), out_names=tuple(out_names),
                lowering_input_output_aliases=(),
                sim_require_finite=True, sim_require_nnan=True, nc=nc))

        n_cores = 8
        devices = jax.devices()[:n_cores]
        mesh = Mesh(np.asarray(devices), ("core",))
        self.sharding = NamedSharding(mesh, PartitionSpec("core"))
        n_out = len(out_names)
        self.fn = jax.jit(
            shard_map(_body, mesh=mesh,
                      in_specs=(PartitionSpec("core"),) * (n_params + n_out),
                      out_specs=(PartitionSpec("core"),) * n_out,
                      check_rep=False),
            donate_argnums=tuple(range(n_params, n_params + n_out)),
            keep_unused=True)
        self.out_avals = out_avals
        self._jax = jax

    def run(self, per_core_inputs):
        """per_core_inputs: list of 8 dicts keyed by bass input name."""
        import jax.numpy as jnp
        jax = self._jax
        concat_in = [
            np.concatenate([np.asarray(per_core_inputs[c][n])
                            for c in range(8)], axis=0)
            for n in self.in_names
        ]
        dev_in = [jax.device_put(a, self.sharding) for a in concat_in]
        zeros = [jnp.zeros((8 * av.shape[0], *av.shape[1:]), av.dtype,
                           device=self.sharding) for av in self.out_avals]
        outs = self.fn(*dev_in, *zeros)
        outs = [np.asarray(o) for o in outs]
        return [
            {n: outs[i].reshape(8, *self.out_avals[i].shape)[c]
             for i, n in enumerate(self.out_names)}
            for c in range(8)
        ]


def get_runner(reps=1, mm_dt="bfloat16"):
    key = (reps, mm_dt)
    if key not in _RUNNER_CACHE:
        _RUNNER_CACHE[key] = _Runner(Cfg(mm_dt=mm_dt, reps=reps))
    return _RUNNER_CACHE[key]


def shard_inputs(x, w_qkv, b_qkv, w_out, b_out, mm_dt="bfloat16"):
    """Full inputs -> 8 per-core input dicts (core c = batch c//2, hgroup c%2)."""
    import ml_dtypes
    mdt = ml_dtypes.bfloat16 if mm_dt == "bfloat16" else np.float32
    x = np.asarray(x, np.float32)
    w_qkv = np.asarray(w_qkv, np.float32)
    b_qkv = np.asarray(b_qkv, np.float32)
    w_out = np.asarray(w_out, np.float32)
    xTs = [np.ascontiguousarray(x[b].T).astype(mdt) for b in range(x.shape[0])]
    per_core = []
    for c in range(8):
        b, hg = c // 2, c % 2
        q_sl = slice(hg * 512, hg * 512 + 512)
        k_sl = slice(1024 + hg * 512, 1024 + hg * 512 + 512)
        v_sl = slice(2048 + hg * 512, 2048 + hg * 512 + 512)
        per_core.append({
            "xt": xTs[b],
            "w_qk": np.ascontiguousarray(
                np.concatenate([w_qkv[:, q_sl], w_qkv[:, k_sl]],
                               axis=1)).astype(mdt),
            "w_v": np.ascontiguousarray(w_qkv[:, v_sl]).astype(mdt),
            "b_qk": np.ascontiguousarray(
                np.concatenate([b_qkv[q_sl], b_qkv[k_sl]])),
            "w_out": np.ascontiguousarray(
                w_out[hg * 512:(hg + 1) * 512, :]).astype(ml_dtypes.bfloat16),
        })
    return per_core


def kernel(x, w_qkv, b_qkv, w_out, b_out):
    runner = get_runner()
    per_core = shard_inputs(x, w_qkv, b_qkv, w_out, b_out)
    results = runner.run(per_core)
    b_v = np.asarray(b_qkv, np.float32)[2048:]
    bias = np.asarray(b_out, np.float32) + b_v @ np.asarray(w_out, np.float32)
    out = np.empty((4, 2048, 1024), np.float32)
    for b in range(4):
        out[b] = (results[2 * b]["yp"].astype(np.float32)
                  + results[2 * b + 1]["yp"].astype(np.float32) + bias)
    return out
